# revision 20
# baseline (speedup 1.0000x reference)
"""Causal self-attention (QK-RMSNorm + RoPE) on 8 Trainium2 NeuronCores.

Problem: x[2,2048,2048], Wq/Wk/Wv/Wo [2048,2048], 16 heads, head_dim 128.

Sharding: core c handles batch b=c//4 and head group g=c%4 (4 heads,
model cols [512g:512g+512)).  Single pass over the host-pre-transposed
xT computes Q, K and V projections (contraction dim on partitions);
q/k get RMS-norm + RoPE fused per i-block and are transposed per head
straight into SBUF-resident qt/kt tiles (no DRAM round trip).  The
k-side RMS scale (merged with 1/sqrt(head_dim)) is folded into the
per-partition `scale` operand of the exp activation in the attention
phase, so kT is stored un-normalized.  Attention uses transposed scores
(eT = exp(scale_k[k] * kT_blk.T @ qT_chunk - 1)), so the AV matmul
(lhsT=v, rhs=eT) directly yields the transposed attention output
yT[d, i] that o_proj consumes.  The softmax denominator comes from a
ones-lhsT matmul over eT; its reciprocal (fast-approx DVE op) is
broadcast across partitions with a K=1 PE matmul.  Per-batch groups of
4 cores AllGather their yT head shards per 512-position chunk (small
replica groups halve collective traffic vs an 8-wide gather), then each
core computes a 512-column slice of the output projection in transposed
layout.  The host de-transposes and concatenates.

Matmuls run on f16 operands with f32 PSUM accumulation.
"""

import math
from contextlib import ExitStack

import numpy as np

import concourse.bass as bass
import concourse.bacc as bacc
import concourse.tile as tile
from concourse import mybir
from concourse.bass_utils import run_bass_kernel_spmd
from concourse.masks import make_identity

P = 128
D = 2048
S = 2048
HD = 128              # head dim
NHL = 4               # heads per core
GW = NHL * HD         # 512, per-core width of head group
CT = D // P           # 16 contraction tiles
ICH = 4               # i-chunks of 512 positions
NCORES = 8
GRP = 4               # cores per batch group (collective size)
F32 = mybir.dt.float32
F16 = mybir.dt.float16
F32R = mybir.dt.float32r
SCALE = 1.0 / math.sqrt(HD)
EPS = 1.1920928955078125e-07
MASK_NEG = -30000.0

_program_cache = {}

# bisection switches for hardware-hang hunting
USE_TTR = False        # fused square+reduce via tensor_tensor_reduce
USE_EXP_SCALE_AP = False  # fold SCALE*rstd_k into exp's per-partition scale


def build_program():
    if "nc" in _program_cache:
        return _program_cache["nc"]

    nc = bacc.Bacc("TRN2", target_bir_lowering=False, debug=False, num_devices=NCORES)

    xt_in = nc.dram_tensor("xt", [D, S], F16, kind="ExternalInput")
    wq_in = nc.dram_tensor("wq", [D, GW], F16, kind="ExternalInput")
    wk_in = nc.dram_tensor("wk", [D, GW], F16, kind="ExternalInput")
    wv_in = nc.dram_tensor("wv", [D, GW], F16, kind="ExternalInput")
    wo_in = nc.dram_tensor("wo", [D, GW], F16, kind="ExternalInput")
    cos_in = nc.dram_tensor("cos", [S, HD // 2], F16, kind="ExternalInput")
    sin_in = nc.dram_tensor("sin", [S, HD // 2], F16, kind="ExternalInput")
    mask_in = nc.dram_tensor("maskt", [4, P, 512], F16, kind="ExternalInput")
    roff_in = nc.dram_tensor("roff", [1, 2], mybir.dt.uint32, kind="ExternalInput")
    yt_out = nc.dram_tensor("yt_out", [GW, S], F32, kind="ExternalOutput")

    with tile.TileContext(nc) as tc:
        with ExitStack() as ctx:
            const = ctx.enter_context(tc.tile_pool(name="const", bufs=1))
            dram = ctx.enter_context(tc.tile_pool(name="dram", bufs=1, space="DRAM"))

            ident = const.tile([P, P], F16, name="ident")
            make_identity(nc, ident)
            eps_t = const.tile([P, 1], F32, name="eps_t")
            nc.vector.memset(eps_t[:], EPS)
            # k-side: sqrt(sumsq + HD*EPS) = sqrt(HD)*sqrt(mean+EPS), so its
            # reciprocal is SCALE * rstd_k directly.
            eps_hd_t = const.tile([P, 1], F32, name="eps_hd_t")
            nc.vector.memset(eps_hd_t[:], EPS * HD)
            neg1_t = const.tile([P, 1], F32, name="neg1_t")
            nc.vector.memset(neg1_t[:], -1.0)
            ones_f = const.tile([P, P], F32, name="ones_f")
            nc.vector.memset(ones_f[:], 1.0)
            ones2 = const.tile([P, 2], F16, name="ones2")
            nc.scalar.copy(ones2[:], ones_f[:, 0:2])

            cos_sb = const.tile([P, CT, HD // 2], F16, name="cos_sb")
            nc.sync.dma_start(out=cos_sb[:], in_=cos_in.ap().rearrange("(a p) f -> p a f", p=P))
            sin_sb = const.tile([P, CT, HD // 2], F16, name="sin_sb")
            nc.sync.dma_start(out=sin_sb[:], in_=sin_in.ap().rearrange("(a p) f -> p a f", p=P))
            mask_sb = const.tile([P, 4, 512], F16, name="mask_sb")
            nc.sync.dma_start(out=mask_sb[:], in_=mask_in.ap().rearrange("t p f -> p t f"))

            yt_ics = [dram.tile([GW, 512], F16, name=f"yt_ic{i}") for i in range(ICH)]
            # 4-core groups can't use Shared-output collectives, and Local
            # output hangs at runtime here, so gather across all 8 cores and
            # dynamic-slice the batch's half in phase D.
            ag_ics = [
                dram.tile([NCORES * GW, 512], F16, name=f"ag_ic{i}", addr_space="Shared")
                for i in range(ICH)
            ]

            # pools that persist from phase A into B/D
            persist = ctx.enter_context(tc.tile_pool(name="persist", bufs=1))
            wo_sb = persist.tile([P, CT, GW], F16, name="wo_sb")
            qt_sb = persist.tile([P, NHL, S], F16, name="qt_sb")
            kt_sb = persist.tile([P, NHL, S], F16, name="kt_sb")
            v_sb = persist.tile([P, CT, GW], F16, name="v_sb")
            # SCALE * rstd_k per (position-block, head), consumed as the exp scale
            rstdk_sb = persist.tile([P, CT, NHL], F32, name="rstdk_sb")

            # ---------------- Phase A: Q, K, V in one x pass ----------------
            with ExitStack() as pha:
                wpool = pha.enter_context(tc.tile_pool(name="wpool", bufs=1))
                xt_pool = pha.enter_context(tc.tile_pool(name="xt_pool", bufs=2))
                proj_ps = pha.enter_context(tc.tile_pool(name="proj_ps", bufs=2, space="PSUM"))
                tp_ps = pha.enter_context(tc.tile_pool(name="tp_ps", bufs=2, space="PSUM"))
                rope = pha.enter_context(tc.tile_pool(name="rope", bufs=2))
                stat = pha.enter_context(tc.tile_pool(name="stat", bufs=2))

                wq_sb = wpool.tile([P, CT, GW], F16, name="wq_sb")
                wk_sb = wpool.tile([P, CT, GW], F16, name="wk_sb")
                wv_sb = wpool.tile([P, CT, GW], F16, name="wv_sb")

                def load_w(dst, src, splits=((0, 8), (8, 16))):
                    # split loads: finer deps + DMA engine parallelism
                    for c0, c1 in splits:
                        nc.sync.dma_start(
                            out=dst[:, c0:c1, :],
                            in_=src.ap()[c0 * P:c1 * P, :]
                                .rearrange("(a p) f -> p a f", p=P),
                        )

                FIRST = ((0, 2), (2, 8), (8, 16))
                load_w(wq_sb, wq_in, FIRST)
                xt_chs = []
                for ica in range(8):
                    xt_ch = xt_pool.tile([P, CT, 256], F16, name=f"xt_ch{ica}", tag="xt")
                    xt_chs.append(xt_ch)
                    for c0, c1 in FIRST if ica == 0 else ((0, 8), (8, 16)):
                        nc.sync.dma_start(
                            out=xt_ch[:, c0:c1, :],
                            in_=xt_in.ap()[c0 * P:c1 * P, ica * 256:(ica + 1) * 256]
                                .rearrange("(a p) f -> p a f", p=P),
                        )
                    if ica == 0:
                        load_w(wk_sb, wk_in)
                        load_w(wv_sb, wv_in)
                        load_w(wo_sb, wo_in)

                    for ib in range(2):
                        ibg = ica * 2 + ib         # global i-block
                        # --- V projection: stays [positions, head-cols] ---
                        vps = proj_ps.tile([P, GW], F32, name=f"vps{ibg}", tag="vps")
                        for ct in range(CT):
                            nc.tensor.matmul(
                                vps[:],
                                xt_ch[:, ct, ib * P:(ib + 1) * P],
                                wv_sb[:, ct, :],
                                start=(ct == 0),
                                stop=(ct == CT - 1),
                            )
                        nc.scalar.copy(v_sb[:, ibg, :], vps[:])

                        # --- Q and K projections ---
                        qps = proj_ps.tile([P, GW], F32, name=f"qps{ibg}", tag="qps")
                        kps = proj_ps.tile([P, GW], F32, name=f"kps{ibg}", tag="kps")
                        for ps, wsb in ((qps, wq_sb), (kps, wk_sb)):
                            for ct in range(CT):
                                nc.tensor.matmul(
                                    ps[:],
                                    xt_ch[:, ct, ib * P:(ib + 1) * P],
                                    wsb[:, ct, :],
                                    start=(ct == 0),
                                    stop=(ct == CT - 1),
                                )
                        # qk: [P, 2, GW] = q then k, processed jointly
                        qk = rope.tile([P, 2, GW], F16, name=f"qk{ibg}", tag="qk")
                        nc.scalar.copy(qk[:, 0, :], qps[:])
                        nc.scalar.copy(qk[:, 1, :], kps[:])

                        # rms-norm stats
                        sq = rope.tile([P, 2, GW], F16, name=f"sq{ibg}", tag="sq")
                        rstd = stat.tile([P, 2 * NHL], F32, name=f"rstd{ibg}", tag="rstd")
                        qk8 = qk[:].rearrange("p a (h d) -> p (a h) d", d=HD)
                        sq8 = sq[:].rearrange("p a (h d) -> p (a h) d", d=HD)
                        if USE_TTR:
                            for j in range(2 * NHL):
                                nc.vector.tensor_tensor_reduce(
                                    out=sq8[:, j, :],
                                    in0=qk8[:, j, :],
                                    in1=qk8[:, j, :],
                                    scale=1.0,
                                    scalar=0.0,
                                    op0=mybir.AluOpType.mult,
                                    op1=mybir.AluOpType.add,
                                    accum_out=rstd[:, j:j + 1],
                                )
                        else:
                            nc.vector.tensor_mul(sq[:], qk[:], qk[:])
                            for j in range(2 * NHL):
                                nc.vector.reduce_sum(
                                    rstd[:, j:j + 1],
                                    sq8[:, j, :],
                                    axis=mybir.AxisListType.X,
                                )
                        # q half: sqrt(mean+eps); k half: sqrt(sumsq+HD*eps)
                        nc.scalar.activation(
                            rstd[:, 0:NHL], rstd[:, 0:NHL],
                            mybir.ActivationFunctionType.Sqrt,
                            bias=eps_t[:], scale=1.0 / HD,
                        )
                        nc.scalar.activation(
                            rstd[:, NHL:2 * NHL], rstd[:, NHL:2 * NHL],
                            mybir.ActivationFunctionType.Sqrt,
                            bias=eps_hd_t[:], scale=1.0,
                        )
                        nc.vector.reciprocal(rstd[:], rstd[:])
                        if USE_EXP_SCALE_AP:
                            nc.scalar.copy(rstdk_sb[:, ibg, :], rstd[:, NHL:2 * NHL])

                        # rope on q and k jointly: 8 (head, q/k) rows of [P, HD]
                        qr = rope.tile([P, 2, GW], F16, name=f"qr{ibg}", tag="qr")
                        qr8 = qr[:].rearrange("p a (h d) -> p (a h) d", d=HD)
                        tmp = rope.tile([P, 2 * NHL, HD // 2], F16, name=f"tmp{ibg}", tag="tmp")
                        cosB = cos_sb[:, ibg:ibg + 1, :].broadcast_to((P, 2 * NHL, HD // 2))
                        sinB = sin_sb[:, ibg:ibg + 1, :].broadcast_to((P, 2 * NHL, HD // 2))
                        h1 = qk8[:, :, 0:HD // 2]
                        h2 = qk8[:, :, HD // 2:HD]
                        # r1 = q1*cos + q2*sin ; r2 = q2*cos - q1*sin
                        nc.vector.tensor_mul(qr8[:, :, 0:HD // 2], h1, cosB)
                        nc.vector.tensor_mul(tmp[:], h2, sinB)
                        nc.vector.tensor_add(qr8[:, :, 0:HD // 2], qr8[:, :, 0:HD // 2], tmp[:])
                        nc.vector.tensor_mul(qr8[:, :, HD // 2:HD], h2, cosB)
                        nc.vector.tensor_mul(tmp[:], h1, sinB)
                        nc.vector.tensor_sub(
                            qr8[:, :, HD // 2:HD], qr8[:, :, HD // 2:HD], tmp[:]
                        )
                        # rstd_q applied to q half; k's either folds into the
                        # exp scale or is applied here (k-half rstd already
                        # includes the 1/sqrt(HD) factor)
                        nmul = NHL if USE_EXP_SCALE_AP else 2 * NHL
                        for j in range(nmul):
                            nc.vector.tensor_scalar_mul(
                                qr8[:, j, :],
                                qr8[:, j, :],
                                rstd[:, j:j + 1],
                            )
                        # transpose per (q/k, head) into SBUF-resident qt/kt
                        for a, dst in ((0, qt_sb), (1, kt_sb)):
                            for h in range(NHL):
                                tp = tp_ps.tile([P, P], F16, name=f"tp{ibg}_{a}_{h}", tag="tp")
                                nc.tensor.transpose(tp[:], qr[:, a, h * HD:(h + 1) * HD], ident[:])
                                nc.scalar.copy(dst[:, h, ibg * P:(ibg + 1) * P], tp[:])

            # ------- Phase B: attention (i-chunk outer) + interleaved o_proj -------
            with ExitStack() as phb:
                et_pool = phb.enter_context(tc.tile_pool(name="et_pool", bufs=4))
                s_ps = phb.enter_context(tc.tile_pool(name="s_ps", bufs=3, space="PSUM"))
                acc_ps = phb.enter_context(tc.tile_pool(name="acc_ps", bufs=2, space="PSUM"))
                den_psp = phb.enter_context(tc.tile_pool(name="den_psp", bufs=2, space="PSUM"))
                bsmall = phb.enter_context(tc.tile_pool(name="bsmall", bufs=2))
                ag_pool = phb.enter_context(tc.tile_pool(name="ag_pool", bufs=2))
                d_ps = phb.enter_context(tc.tile_pool(name="d_ps", bufs=1, space="PSUM"))
                ev2 = phb.enter_context(tc.tile_pool(name="ev2", bufs=3))
                roffp = phb.enter_context(tc.tile_pool(name="roffp", bufs=1))

                roff_sb = roffp.tile([1, 2], mybir.dt.uint32, name="roff_sb")
                nc.sync.dma_start(out=roff_sb[:], in_=roff_in[:, :])
                roff_reg = nc.alloc_registers()
                nc.regs_load(roff_reg, roff_sb[0:1, 0:1])
                rv = nc.snap(roff_reg, donate=True)

                def emit_score(ic, h, jb):
                    """score matmul + exp + causal mask for one j-block."""
                    sp = s_ps.tile([P, 512], F32, name=f"s{h}_{ic}_{jb}", tag="s")
                    nc.tensor.matmul(
                        sp[:],
                        kt_sb[:, h, jb * P:(jb + 1) * P],
                        qt_sb[:, h, ic * 512:(ic + 1) * 512],
                        start=True, stop=True,
                    )
                    et = et_pool.tile([P, 512], F16, name=f"et{h}_{ic}_{jb}", tag="et")
                    nc.scalar.activation(
                        et[:], sp[:],
                        mybir.ActivationFunctionType.Exp,
                        bias=neg1_t[:],
                        scale=rstdk_sb[:, jb, h:h + 1] if USE_EXP_SCALE_AP else 1.0,
                    )
                    t = jb - 4 * ic
                    if t >= 0:
                        nc.vector.tensor_mul(et[:], et[:], mask_sb[:, t, :])
                    return et

                def emit_oproj(icc):
                    """o_proj for one gathered 512-position chunk."""
                    ag_ch = ag_pool.tile([P, CT, 512], F16, name=f"ag{icc}", tag="ag")
                    # 4 quarter-loads: spread across DMA engines, finer deps
                    for quart in range(4):
                        c0, c1 = quart * (CT // 4), (quart + 1) * (CT // 4)
                        nc.sync.dma_start(
                            out=ag_ch[:, c0:c1, :],
                            in_=ag_ics[icc][bass.ds(rv, D), :]
                                .rearrange("(t p) f -> p t f", p=P)[:, c0:c1, :],
                        )
                    for oc in range(4):
                        y_ps = d_ps.tile([P, 512], F32, name=f"yp{icc}_{oc}", tag="yp")
                        for mt in range(CT):
                            nc.tensor.matmul(
                                y_ps[:],
                                wo_sb[:, mt, oc * P:(oc + 1) * P],
                                ag_ch[:, mt, :],
                                start=(mt == 0), stop=(mt == CT - 1),
                            )
                        y_sb = ev2.tile([P, 512], F32, name=f"ysb{icc}_{oc}", tag="ysb")
                        nc.scalar.copy(y_sb[:], y_ps[:])
                        nc.scalar.dma_start(
                            out=yt_out[oc * P:(oc + 1) * P, icc * 512:(icc + 1) * 512],
                            in_=y_sb[:],
                        )

                for ic in range(ICH):
                    njb = 4 * ic + 4
                    for h in range(NHL):
                        yt_ps = acc_ps.tile([P, 512], F32, name=f"yt{h}_{ic}", tag="yt")
                        den_ps = den_psp.tile([2, 512], F32, name=f"den{h}_{ic}", tag="den")
                        # software-pipelined: scores run 2 j-blocks ahead so the
                        # PE never head-of-line blocks on exp(jb)
                        ets = {0: emit_score(ic, h, 0)}
                        if njb > 1:
                            ets[1] = emit_score(ic, h, 1)
                        for jb in range(njb):
                            if jb + 2 < njb:
                                ets[jb + 2] = emit_score(ic, h, jb + 2)
                            et = ets.pop(jb)
                            nc.tensor.matmul(
                                yt_ps[:],
                                v_sb[:, jb, h * HD:(h + 1) * HD],
                                et[:],
                                start=(jb == 0), stop=(jb == njb - 1),
                            )
                            nc.tensor.matmul(
                                den_ps[:],
                                ones2[:],
                                et[:],
                                start=(jb == 0), stop=(jb == njb - 1),
                            )
                        rden = bsmall.tile([1, 512], F32, name=f"rd{h}_{ic}", tag="rden")
                        nc.vector.reciprocal_approx_fast(rden[:], den_ps[0:1, :])
                        # broadcast 1/den across partitions on the idle gpsimd
                        # engine (no PE matmul, no PSUM bank)
                        bc_sb = bsmall.tile([P, 512], F32, name=f"bcs{h}_{ic}", tag="bcs")
                        nc.gpsimd.partition_broadcast(bc_sb[:], rden[:], channels=P)
                        yt_sb = bsmall.tile([P, 512], F16, name=f"yts{h}_{ic}", tag="yts")
                        nc.vector.tensor_mul(yt_sb[:], yt_ps[:], bc_sb[:])
                        nc.gpsimd.dma_start(
                            out=yt_ics[ic][h * P:(h + 1) * P, :],
                            in_=yt_sb[:],
                        )
                    # per-chunk AllGather; fires as soon as chunk ic is written
                    nc.gpsimd.collective_compute(
                        "AllGather",
                        mybir.AluOpType.bypass,
                        replica_groups=[list(range(NCORES))],
                        ins=[yt_ics[ic][:].opt()],
                        outs=[ag_ics[ic][:].opt()],
                    )
                    # o_proj for chunk ic-2: its AllGather completed during the
                    # previous attention chunk, so no PE head-of-line stall
                    if ic >= 2:
                        emit_oproj(ic - 2)
                emit_oproj(ICH - 2)
                emit_oproj(ICH - 1)

    nc.compile()
    _program_cache["nc"] = nc
    return nc


def _rope_tables():
    inv_freq = 1.0 / (10000.0 ** (np.arange(0, HD, 2, dtype=np.float32) / HD))
    pos = np.arange(S, dtype=np.float32)
    freqs = np.outer(pos, inv_freq).astype(np.float32)
    return np.cos(freqs).astype(np.float16), np.sin(freqs).astype(np.float16)


def _mask_tiles():
    m = np.zeros((4, P, 512), dtype=np.float16)
    jj = np.arange(P)[:, None]
    ii = np.arange(512)[None, :]
    for t in range(4):
        m[t] = np.where(t * P + jj > ii, 0.0, 1.0)
    return m


def make_in_maps(x, Wq, Wk, Wv, Wo):
    x = np.asarray(x, dtype=np.float32)
    cos, sin = _rope_tables()
    maskt = _mask_tiles()
    wqT = np.ascontiguousarray(np.asarray(Wq, dtype=np.float32).T.astype(np.float16))
    wkT = np.ascontiguousarray(np.asarray(Wk, dtype=np.float32).T.astype(np.float16))
    wvT = np.ascontiguousarray(np.asarray(Wv, dtype=np.float32).T.astype(np.float16))
    woT = np.ascontiguousarray(np.asarray(Wo, dtype=np.float32).T.astype(np.float16))
    xts = [np.ascontiguousarray(x[b].T.astype(np.float16)) for b in range(2)]
    in_maps = []
    for c in range(NCORES):
        b, g = c // 4, c % 4
        sl = slice(g * GW, (g + 1) * GW)
        in_maps.append({
            "roff": np.array([[b * D, 0]], dtype=np.uint32),
            "xt": xts[b],
            "wq": np.ascontiguousarray(wqT[:, sl]),
            "wk": np.ascontiguousarray(wkT[:, sl]),
            "wv": np.ascontiguousarray(wvT[:, sl]),
            "wo": np.ascontiguousarray(woT[:, sl]),
            "cos": cos,
            "sin": sin,
            "maskt": maskt,
        })
    return in_maps


def assemble_output(results):
    y = np.empty((2, S, D), dtype=np.float32)
    for c in range(NCORES):
        b, g = c // 4, c % 4
        y[b][:, g * GW:(g + 1) * GW] = results[c]["yt_out"].T
    return y


def kernel(x, Wq, Wk, Wv, Wo):
    nc = build_program()
    in_maps = make_in_maps(x, Wq, Wk, Wv, Wo)
    res = run_bass_kernel_spmd(nc, in_maps, core_ids=list(range(NCORES)))
    return assemble_output(res.results)


# revision 25
# speedup vs baseline: 1.0110x; 1.0110x over previous
"""Causal self-attention (QK-RMSNorm + RoPE) on 8 Trainium2 NeuronCores.

Problem: x[2,2048,2048], Wq/Wk/Wv/Wo [2048,2048], 16 heads, head_dim 128.

Sharding: core c handles batch b=c//4 and head group g=c%4 (4 heads,
model cols [512g:512g+512)).  Single pass over the host-pre-transposed
xT computes Q, K and V projections (contraction dim on partitions);
q/k get RMS-norm + RoPE fused per i-block and are transposed per head
straight into SBUF-resident qt/kt tiles (no DRAM round trip).  The
k-side RMS scale (merged with 1/sqrt(head_dim)) is folded into the
per-partition `scale` operand of the exp activation in the attention
phase, so kT is stored un-normalized.  Attention uses transposed scores
(eT = exp(scale_k[k] * kT_blk.T @ qT_chunk - 1)), so the AV matmul
(lhsT=v, rhs=eT) directly yields the transposed attention output
yT[d, i] that o_proj consumes.  The softmax denominator comes from a
ones-lhsT matmul over eT; its reciprocal (fast-approx DVE op) is
broadcast across partitions with a K=1 PE matmul.  Per-batch groups of
4 cores AllGather their yT head shards per 512-position chunk (small
replica groups halve collective traffic vs an 8-wide gather), then each
core computes a 512-column slice of the output projection in transposed
layout.  The host de-transposes and concatenates.

Matmuls run on f16 operands with f32 PSUM accumulation.
"""

import math
from contextlib import ExitStack

import numpy as np

import concourse.bass as bass
import concourse.bacc as bacc
import concourse.tile as tile
from concourse import mybir
from concourse.bass_utils import run_bass_kernel_spmd
from concourse.masks import make_identity

P = 128
D = 2048
S = 2048
HD = 128              # head dim
NHL = 4               # heads per core
GW = NHL * HD         # 512, per-core width of head group
CT = D // P           # 16 contraction tiles
ICH = 4               # i-chunks of 512 positions
NCORES = 8
GRP = 4               # cores per batch group (collective size)
F32 = mybir.dt.float32
F16 = mybir.dt.float16
F32R = mybir.dt.float32r
SCALE = 1.0 / math.sqrt(HD)
EPS = 1.1920928955078125e-07
MASK_NEG = -30000.0

_program_cache = {}

# bisection switches for hardware-hang hunting
USE_TTR = False        # fused square+reduce via tensor_tensor_reduce
USE_EXP_SCALE_AP = False  # fold SCALE*rstd_k into exp's per-partition scale


def build_program():
    if "nc" in _program_cache:
        return _program_cache["nc"]

    nc = bacc.Bacc("TRN2", target_bir_lowering=False, debug=False, num_devices=NCORES)

    xt_in = nc.dram_tensor("xt", [D, S], F16, kind="ExternalInput")
    wq_in = nc.dram_tensor("wq", [D, GW], F16, kind="ExternalInput")
    wk_in = nc.dram_tensor("wk", [D, GW], F16, kind="ExternalInput")
    wv_in = nc.dram_tensor("wv", [D, GW], F16, kind="ExternalInput")
    wo_in = nc.dram_tensor("wo", [D, GW], F16, kind="ExternalInput")
    cos_in = nc.dram_tensor("cos", [S, HD // 2], F16, kind="ExternalInput")
    sin_in = nc.dram_tensor("sin", [S, HD // 2], F16, kind="ExternalInput")
    mask_in = nc.dram_tensor("maskt", [4, P, 512], F16, kind="ExternalInput")
    roff_in = nc.dram_tensor("roff", [1, 2], mybir.dt.uint32, kind="ExternalInput")
    yt_out = nc.dram_tensor("yt_out", [GW, S], F32, kind="ExternalOutput")

    with tile.TileContext(nc) as tc:
        with ExitStack() as ctx:
            const = ctx.enter_context(tc.tile_pool(name="const", bufs=1))
            dram = ctx.enter_context(tc.tile_pool(name="dram", bufs=1, space="DRAM"))

            ident = const.tile([P, P], F16, name="ident")
            make_identity(nc, ident)
            eps_t = const.tile([P, 1], F32, name="eps_t")
            nc.vector.memset(eps_t[:], EPS)
            # k-side: sqrt(sumsq + HD*EPS) = sqrt(HD)*sqrt(mean+EPS), so its
            # reciprocal is SCALE * rstd_k directly.
            eps_hd_t = const.tile([P, 1], F32, name="eps_hd_t")
            nc.vector.memset(eps_hd_t[:], EPS * HD)
            neg1_t = const.tile([P, 1], F32, name="neg1_t")
            nc.vector.memset(neg1_t[:], -1.0)
            ones_f = const.tile([P, P], F32, name="ones_f")
            nc.vector.memset(ones_f[:], 1.0)
            ones2 = const.tile([P, 2], F16, name="ones2")
            nc.scalar.copy(ones2[:], ones_f[:, 0:2])

            # cos/sin/mask tiles declared here; DMAs issued inside phase A
            # after the startup-critical wq/xt pieces
            cos_sb = const.tile([P, CT, HD // 2], F16, name="cos_sb")
            sin_sb = const.tile([P, CT, HD // 2], F16, name="sin_sb")
            mask_sb = const.tile([P, 4, 512], F16, name="mask_sb")

            yt_ics = [dram.tile([GW, 512], F16, name=f"yt_ic{i}") for i in range(ICH)]
            # 4-core groups can't use Shared-output collectives, and Local
            # output hangs at runtime here, so gather across all 8 cores and
            # dynamic-slice the batch's half in phase D.
            ag_ics = [
                dram.tile([NCORES * GW, 512], F16, name=f"ag_ic{i}", addr_space="Shared")
                for i in range(ICH)
            ]

            # pools that persist from phase A into B/D
            persist = ctx.enter_context(tc.tile_pool(name="persist", bufs=1))
            wo_sb = persist.tile([P, CT, GW], F16, name="wo_sb")
            qt_sb = persist.tile([P, NHL, S], F16, name="qt_sb")
            kt_sb = persist.tile([P, NHL, S], F16, name="kt_sb")
            v_sb = persist.tile([P, CT, GW], F16, name="v_sb")
            # SCALE * rstd_k per (position-block, head), consumed as the exp scale
            rstdk_sb = persist.tile([P, CT, NHL], F32, name="rstdk_sb")

            # ---------------- Phase A: Q, K, V in one x pass ----------------
            with ExitStack() as pha:
                wpool = pha.enter_context(tc.tile_pool(name="wpool", bufs=1))
                xt_pool = pha.enter_context(tc.tile_pool(name="xt_pool", bufs=4))
                proj_ps = pha.enter_context(tc.tile_pool(name="proj_ps", bufs=2, space="PSUM"))
                tp_ps = pha.enter_context(tc.tile_pool(name="tp_ps", bufs=2, space="PSUM"))
                rope = pha.enter_context(tc.tile_pool(name="rope", bufs=2))
                stat = pha.enter_context(tc.tile_pool(name="stat", bufs=2))

                wq_sb = wpool.tile([P, CT, GW], F16, name="wq_sb")
                wk_sb = wpool.tile([P, CT, GW], F16, name="wk_sb")
                wv_sb = wpool.tile([P, CT, GW], F16, name="wv_sb")

                def load_w(dst, src, splits=((0, 8), (8, 16))):
                    # split loads: finer deps + DMA engine parallelism
                    for c0, c1 in splits:
                        nc.sync.dma_start(
                            out=dst[:, c0:c1, :],
                            in_=src.ap()[c0 * P:c1 * P, :]
                                .rearrange("(a p) f -> p a f", p=P),
                        )

                FIRST = ((0, 1), (1, 4), (4, 10), (10, 16))
                QUARTS = ((0, 4), (4, 8), (8, 12), (12, 16))

                def load_xt(xt_ch, ica, splits):
                    for c0, c1 in splits:
                        nc.sync.dma_start(
                            out=xt_ch[:, c0:c1, :],
                            in_=xt_in.ap()[c0 * P:c1 * P, ica * 256:(ica + 1) * 256]
                                .rearrange("(a p) f -> p a f", p=P),
                        )

                # startup-critical interleave: first wq piece, first xt piece,
                # first cos/sin rows, then the rest
                nc.sync.dma_start(
                    out=wq_sb[:, 0:1, :],
                    in_=wq_in.ap()[0:P, :].rearrange("(a p) f -> p a f", p=P))
                xt_chs = []
                xt0 = xt_pool.tile([P, CT, 256], F16, name="xt_ch0", tag="xt")
                xt_chs.append(xt0)
                nc.sync.dma_start(
                    out=xt0[:, 0:1, :],
                    in_=xt_in.ap()[0:P, 0:256].rearrange("(a p) f -> p a f", p=P))
                nc.sync.dma_start(
                    out=cos_sb[:, 0:2, :],
                    in_=cos_in.ap()[0:2 * P, :].rearrange("(a p) f -> p a f", p=P))
                nc.sync.dma_start(
                    out=sin_sb[:, 0:2, :],
                    in_=sin_in.ap()[0:2 * P, :].rearrange("(a p) f -> p a f", p=P))
                load_w(wq_sb, wq_in, ((1, 4), (4, 10), (10, 16)))
                load_xt(xt0, 0, ((1, 4), (4, 10), (10, 16)))
                nc.sync.dma_start(
                    out=cos_sb[:, 2:CT, :],
                    in_=cos_in.ap()[2 * P:CT * P, :].rearrange("(a p) f -> p a f", p=P))
                nc.sync.dma_start(
                    out=sin_sb[:, 2:CT, :],
                    in_=sin_in.ap()[2 * P:CT * P, :].rearrange("(a p) f -> p a f", p=P))

                for ica in range(8):
                    if ica > 0:
                        xt_ch = xt_pool.tile([P, CT, 256], F16, name=f"xt_ch{ica}", tag="xt")
                        xt_chs.append(xt_ch)
                        load_xt(xt_ch, ica, QUARTS)
                    else:
                        xt_ch = xt0
                    if ica == 0:
                        load_w(wk_sb, wk_in)
                        load_w(wv_sb, wv_in)
                        load_w(wo_sb, wo_in)
                        nc.sync.dma_start(
                            out=mask_sb[:], in_=mask_in.ap().rearrange("t p f -> p t f"))

                    for ib in range(2):
                        ibg = ica * 2 + ib         # global i-block
                        # --- V projection: stays [positions, head-cols] ---
                        vps = proj_ps.tile([P, GW], F32, name=f"vps{ibg}", tag="vps")
                        for ct in range(CT):
                            nc.tensor.matmul(
                                vps[:],
                                xt_ch[:, ct, ib * P:(ib + 1) * P],
                                wv_sb[:, ct, :],
                                start=(ct == 0),
                                stop=(ct == CT - 1),
                            )
                        nc.scalar.copy(v_sb[:, ibg, :], vps[:])

                        # --- Q and K projections ---
                        qps = proj_ps.tile([P, GW], F32, name=f"qps{ibg}", tag="qps")
                        kps = proj_ps.tile([P, GW], F32, name=f"kps{ibg}", tag="kps")
                        for ps, wsb in ((qps, wq_sb), (kps, wk_sb)):
                            for ct in range(CT):
                                nc.tensor.matmul(
                                    ps[:],
                                    xt_ch[:, ct, ib * P:(ib + 1) * P],
                                    wsb[:, ct, :],
                                    start=(ct == 0),
                                    stop=(ct == CT - 1),
                                )
                        # qk: [P, 2, GW] = q then k, processed jointly
                        qk = rope.tile([P, 2, GW], F16, name=f"qk{ibg}", tag="qk")
                        nc.scalar.copy(qk[:, 0, :], qps[:])
                        nc.scalar.copy(qk[:, 1, :], kps[:])

                        # rms-norm stats
                        sq = rope.tile([P, 2, GW], F16, name=f"sq{ibg}", tag="sq")
                        rstd = stat.tile([P, 2 * NHL], F32, name=f"rstd{ibg}", tag="rstd")
                        qk8 = qk[:].rearrange("p a (h d) -> p (a h) d", d=HD)
                        sq8 = sq[:].rearrange("p a (h d) -> p (a h) d", d=HD)
                        if USE_TTR:
                            for j in range(2 * NHL):
                                nc.vector.tensor_tensor_reduce(
                                    out=sq8[:, j, :],
                                    in0=qk8[:, j, :],
                                    in1=qk8[:, j, :],
                                    scale=1.0,
                                    scalar=0.0,
                                    op0=mybir.AluOpType.mult,
                                    op1=mybir.AluOpType.add,
                                    accum_out=rstd[:, j:j + 1],
                                )
                        else:
                            nc.vector.tensor_mul(sq[:], qk[:], qk[:])
                            for j in range(2 * NHL):
                                nc.vector.reduce_sum(
                                    rstd[:, j:j + 1],
                                    sq8[:, j, :],
                                    axis=mybir.AxisListType.X,
                                )
                        # q half: sqrt(mean+eps); k half: sqrt(sumsq+HD*eps)
                        nc.scalar.activation(
                            rstd[:, 0:NHL], rstd[:, 0:NHL],
                            mybir.ActivationFunctionType.Sqrt,
                            bias=eps_t[:], scale=1.0 / HD,
                        )
                        nc.scalar.activation(
                            rstd[:, NHL:2 * NHL], rstd[:, NHL:2 * NHL],
                            mybir.ActivationFunctionType.Sqrt,
                            bias=eps_hd_t[:], scale=1.0,
                        )
                        nc.vector.reciprocal(rstd[:], rstd[:])
                        if USE_EXP_SCALE_AP:
                            nc.scalar.copy(rstdk_sb[:, ibg, :], rstd[:, NHL:2 * NHL])

                        # rope on q and k jointly: 8 (head, q/k) rows of [P, HD]
                        qr = rope.tile([P, 2, GW], F16, name=f"qr{ibg}", tag="qr")
                        qr8 = qr[:].rearrange("p a (h d) -> p (a h) d", d=HD)
                        tmp = rope.tile([P, 2 * NHL, HD // 2], F16, name=f"tmp{ibg}", tag="tmp")
                        cosB = cos_sb[:, ibg:ibg + 1, :].broadcast_to((P, 2 * NHL, HD // 2))
                        sinB = sin_sb[:, ibg:ibg + 1, :].broadcast_to((P, 2 * NHL, HD // 2))
                        h1 = qk8[:, :, 0:HD // 2]
                        h2 = qk8[:, :, HD // 2:HD]
                        # r1 = q1*cos + q2*sin ; r2 = q2*cos - q1*sin
                        nc.vector.tensor_mul(qr8[:, :, 0:HD // 2], h1, cosB)
                        nc.vector.tensor_mul(tmp[:], h2, sinB)
                        nc.vector.tensor_add(qr8[:, :, 0:HD // 2], qr8[:, :, 0:HD // 2], tmp[:])
                        nc.vector.tensor_mul(qr8[:, :, HD // 2:HD], h2, cosB)
                        nc.vector.tensor_mul(tmp[:], h1, sinB)
                        nc.vector.tensor_sub(
                            qr8[:, :, HD // 2:HD], qr8[:, :, HD // 2:HD], tmp[:]
                        )
                        # rstd_q applied to q half; k's either folds into the
                        # exp scale or is applied here (k-half rstd already
                        # includes the 1/sqrt(HD) factor)
                        nmul = NHL if USE_EXP_SCALE_AP else 2 * NHL
                        for j in range(nmul):
                            nc.vector.tensor_scalar_mul(
                                qr8[:, j, :],
                                qr8[:, j, :],
                                rstd[:, j:j + 1],
                            )
                        # transpose per (q/k, head) into SBUF-resident qt/kt
                        for a, dst in ((0, qt_sb), (1, kt_sb)):
                            for h in range(NHL):
                                tp = tp_ps.tile([P, P], F16, name=f"tp{ibg}_{a}_{h}", tag="tp")
                                nc.tensor.transpose(tp[:], qr[:, a, h * HD:(h + 1) * HD], ident[:])
                                nc.scalar.copy(dst[:, h, ibg * P:(ibg + 1) * P], tp[:])

            # ------- Phase B: attention (i-chunk outer) + interleaved o_proj -------
            with ExitStack() as phb:
                et_pool = phb.enter_context(tc.tile_pool(name="et_pool", bufs=6))
                s_ps = phb.enter_context(tc.tile_pool(name="s_ps", bufs=3, space="PSUM"))
                acc_ps = phb.enter_context(tc.tile_pool(name="acc_ps", bufs=2, space="PSUM"))
                den_psp = phb.enter_context(tc.tile_pool(name="den_psp", bufs=2, space="PSUM"))
                bsmall = phb.enter_context(tc.tile_pool(name="bsmall", bufs=2))
                ag_pool = phb.enter_context(tc.tile_pool(name="ag_pool", bufs=2))
                d_ps = phb.enter_context(tc.tile_pool(name="d_ps", bufs=1, space="PSUM"))
                ev2 = phb.enter_context(tc.tile_pool(name="ev2", bufs=3))
                roffp = phb.enter_context(tc.tile_pool(name="roffp", bufs=1))

                roff_sb = roffp.tile([1, 2], mybir.dt.uint32, name="roff_sb")
                nc.sync.dma_start(out=roff_sb[:], in_=roff_in[:, :])
                roff_reg = nc.alloc_registers()
                nc.regs_load(roff_reg, roff_sb[0:1, 0:1])
                rv = nc.snap(roff_reg, donate=True)

                def emit_score(ic, h, jb):
                    """score matmul + exp + causal mask for one j-block."""
                    sp = s_ps.tile([P, 512], F32, name=f"s{h}_{ic}_{jb}", tag="s")
                    nc.tensor.matmul(
                        sp[:],
                        kt_sb[:, h, jb * P:(jb + 1) * P],
                        qt_sb[:, h, ic * 512:(ic + 1) * 512],
                        start=True, stop=True,
                    )
                    et = et_pool.tile([P, 512], F16, name=f"et{h}_{ic}_{jb}", tag="et")
                    nc.scalar.activation(
                        et[:], sp[:],
                        mybir.ActivationFunctionType.Exp,
                        bias=neg1_t[:],
                        scale=rstdk_sb[:, jb, h:h + 1] if USE_EXP_SCALE_AP else 1.0,
                    )
                    t = jb - 4 * ic
                    if t >= 0:
                        nc.vector.tensor_mul(et[:], et[:], mask_sb[:, t, :])
                    return et

                def emit_oproj(icc):
                    """o_proj for one gathered 512-position chunk."""
                    ag_ch = ag_pool.tile([P, CT, 512], F16, name=f"ag{icc}", tag="ag")
                    # 4 quarter-loads: spread across DMA engines, finer deps
                    for quart in range(4):
                        c0, c1 = quart * (CT // 4), (quart + 1) * (CT // 4)
                        nc.sync.dma_start(
                            out=ag_ch[:, c0:c1, :],
                            in_=ag_ics[icc][bass.ds(rv, D), :]
                                .rearrange("(t p) f -> p t f", p=P)[:, c0:c1, :],
                        )
                    for oc in range(4):
                        y_ps = d_ps.tile([P, 512], F32, name=f"yp{icc}_{oc}", tag="yp")
                        for mt in range(CT):
                            nc.tensor.matmul(
                                y_ps[:],
                                wo_sb[:, mt, oc * P:(oc + 1) * P],
                                ag_ch[:, mt, :],
                                start=(mt == 0), stop=(mt == CT - 1),
                            )
                        y_sb = ev2.tile([P, 512], F32, name=f"ysb{icc}_{oc}", tag="ysb")
                        nc.scalar.copy(y_sb[:], y_ps[:])
                        nc.scalar.dma_start(
                            out=yt_out[oc * P:(oc + 1) * P, icc * 512:(icc + 1) * 512],
                            in_=y_sb[:],
                        )

                for ic in range(ICH):
                    njb = 4 * ic + 4
                    for h in range(NHL):
                        yt_ps = acc_ps.tile([P, 512], F32, name=f"yt{h}_{ic}", tag="yt")
                        den_ps = den_psp.tile([2, 512], F32, name=f"den{h}_{ic}", tag="den")
                        # software-pipelined: scores run 2 j-blocks ahead so the
                        # PE never head-of-line blocks on exp(jb)
                        ets = {0: emit_score(ic, h, 0)}
                        if njb > 1:
                            ets[1] = emit_score(ic, h, 1)
                        for jb in range(njb):
                            if jb + 2 < njb:
                                ets[jb + 2] = emit_score(ic, h, jb + 2)
                            et = ets.pop(jb)
                            nc.tensor.matmul(
                                yt_ps[:],
                                v_sb[:, jb, h * HD:(h + 1) * HD],
                                et[:],
                                start=(jb == 0), stop=(jb == njb - 1),
                            )
                            nc.tensor.matmul(
                                den_ps[:],
                                ones2[:],
                                et[:],
                                start=(jb == 0), stop=(jb == njb - 1),
                            )
                        rden = bsmall.tile([1, 512], F32, name=f"rd{h}_{ic}", tag="rden")
                        nc.vector.reciprocal_approx_fast(rden[:], den_ps[0:1, :])
                        # broadcast 1/den across partitions on the idle gpsimd
                        # engine (no PE matmul, no PSUM bank)
                        bc_sb = bsmall.tile([P, 512], F32, name=f"bcs{h}_{ic}", tag="bcs")
                        nc.gpsimd.partition_broadcast(bc_sb[:], rden[:], channels=P)
                        yt_sb = bsmall.tile([P, 512], F16, name=f"yts{h}_{ic}", tag="yts")
                        nc.vector.tensor_mul(yt_sb[:], yt_ps[:], bc_sb[:])
                        nc.gpsimd.dma_start(
                            out=yt_ics[ic][h * P:(h + 1) * P, :],
                            in_=yt_sb[:],
                        )
                    # per-chunk AllGather; fires as soon as chunk ic is written
                    nc.gpsimd.collective_compute(
                        "AllGather",
                        mybir.AluOpType.bypass,
                        replica_groups=[list(range(NCORES))],
                        ins=[yt_ics[ic][:].opt()],
                        outs=[ag_ics[ic][:].opt()],
                    )
                    # o_proj for chunk ic-2: its AllGather completed during the
                    # previous attention chunk, so no PE head-of-line stall
                    if ic >= 2:
                        with tc.tile_wait_until(0.26 + 0.03 * (ic - 2)):
                            emit_oproj(ic - 2)
                with tc.tile_wait_until(0.32):
                    emit_oproj(ICH - 2)
                with tc.tile_wait_until(0.36):
                    emit_oproj(ICH - 1)

    nc.compile()
    _program_cache["nc"] = nc
    return nc


def _rope_tables():
    inv_freq = 1.0 / (10000.0 ** (np.arange(0, HD, 2, dtype=np.float32) / HD))
    pos = np.arange(S, dtype=np.float32)
    freqs = np.outer(pos, inv_freq).astype(np.float32)
    return np.cos(freqs).astype(np.float16), np.sin(freqs).astype(np.float16)


def _mask_tiles():
    m = np.zeros((4, P, 512), dtype=np.float16)
    jj = np.arange(P)[:, None]
    ii = np.arange(512)[None, :]
    for t in range(4):
        m[t] = np.where(t * P + jj > ii, 0.0, 1.0)
    return m


def make_in_maps(x, Wq, Wk, Wv, Wo):
    x = np.asarray(x, dtype=np.float32)
    cos, sin = _rope_tables()
    maskt = _mask_tiles()
    wqT = np.ascontiguousarray(np.asarray(Wq, dtype=np.float32).T.astype(np.float16))
    wkT = np.ascontiguousarray(np.asarray(Wk, dtype=np.float32).T.astype(np.float16))
    wvT = np.ascontiguousarray(np.asarray(Wv, dtype=np.float32).T.astype(np.float16))
    woT = np.ascontiguousarray(np.asarray(Wo, dtype=np.float32).T.astype(np.float16))
    xts = [np.ascontiguousarray(x[b].T.astype(np.float16)) for b in range(2)]
    in_maps = []
    for c in range(NCORES):
        b, g = c // 4, c % 4
        sl = slice(g * GW, (g + 1) * GW)
        in_maps.append({
            "roff": np.array([[b * D, 0]], dtype=np.uint32),
            "xt": xts[b],
            "wq": np.ascontiguousarray(wqT[:, sl]),
            "wk": np.ascontiguousarray(wkT[:, sl]),
            "wv": np.ascontiguousarray(wvT[:, sl]),
            "wo": np.ascontiguousarray(woT[:, sl]),
            "cos": cos,
            "sin": sin,
            "maskt": maskt,
        })
    return in_maps


def assemble_output(results):
    y = np.empty((2, S, D), dtype=np.float32)
    for c in range(NCORES):
        b, g = c // 4, c % 4
        y[b][:, g * GW:(g + 1) * GW] = results[c]["yt_out"].T
    return y


def kernel(x, Wq, Wk, Wv, Wo):
    nc = build_program()
    in_maps = make_in_maps(x, Wq, Wk, Wv, Wo)
    res = run_bass_kernel_spmd(nc, in_maps, core_ids=list(range(NCORES)))
    return assemble_output(res.results)


# revision 35
# speedup vs baseline: 1.0533x; 1.0418x over previous
"""Causal self-attention (QK-RMSNorm + RoPE) on 8 Trainium2 NeuronCores.

Problem: x[2,2048,2048], Wq/Wk/Wv/Wo [2048,2048], 16 heads, head_dim 128.

Sharding: core c handles batch b=c//4 and head group g=c%4 (4 heads,
model cols [512g:512g+512)).  Single pass over the host-pre-transposed
xT computes Q, K and V projections (contraction dim on partitions);
q/k get RMS-norm + RoPE fused per i-block and are transposed per head
straight into SBUF-resident qt/kt tiles (no DRAM round trip).  The
k-side RMS scale (merged with 1/sqrt(head_dim)) is folded into the
per-partition `scale` operand of the exp activation in the attention
phase, so kT is stored un-normalized.  Attention uses transposed scores
(eT = exp(scale_k[k] * kT_blk.T @ qT_chunk - 1)), so the AV matmul
(lhsT=v, rhs=eT) directly yields the transposed attention output
yT[d, i] that o_proj consumes.  The softmax denominator comes from a
ones-lhsT matmul over eT; its reciprocal (fast-approx DVE op) is
broadcast across partitions with a K=1 PE matmul.  Per-batch groups of
4 cores AllGather their yT head shards per 512-position chunk (small
replica groups halve collective traffic vs an 8-wide gather), then each
core computes a 512-column slice of the output projection in transposed
layout.  The host de-transposes and concatenates.

Matmuls run on f16 operands with f32 PSUM accumulation.
"""

import math
from contextlib import ExitStack

import numpy as np

import concourse.bass as bass
import concourse.bacc as bacc
import concourse.tile as tile
from concourse import mybir
from concourse.bass_utils import run_bass_kernel_spmd
from concourse.masks import make_identity

P = 128
D = 2048
S = 2048
HD = 128              # head dim
NHL = 4               # heads per core
GW = NHL * HD         # 512, per-core width of head group
CT = D // P           # 16 contraction tiles
ICH = 4               # i-chunks of 512 positions
NCORES = 8
GRP = 4               # cores per batch group (collective size)
F32 = mybir.dt.float32
F16 = mybir.dt.float16
F32R = mybir.dt.float32r
SCALE = 1.0 / math.sqrt(HD)
EPS = 1.1920928955078125e-07
MASK_NEG = -30000.0

_program_cache = {}

# bisection switches for hardware-hang hunting
USE_TTR = False        # fused square+reduce via tensor_tensor_reduce
USE_EXP_SCALE_AP = False  # fold SCALE*rstd_k into exp's per-partition scale


def build_program():
    if "nc" in _program_cache:
        return _program_cache["nc"]

    nc = bacc.Bacc("TRN2", target_bir_lowering=False, debug=False, num_devices=NCORES)

    xt_in = nc.dram_tensor("xt", [D, S], F16, kind="ExternalInput")
    wq_in = nc.dram_tensor("wq", [D, GW], F16, kind="ExternalInput")
    wk_in = nc.dram_tensor("wk", [D, GW], F16, kind="ExternalInput")
    wv_in = nc.dram_tensor("wv", [D, GW], F16, kind="ExternalInput")
    wo_in = nc.dram_tensor("wo", [D, GW], F16, kind="ExternalInput")
    cos_in = nc.dram_tensor("cos", [S, HD // 2], F16, kind="ExternalInput")
    sin_in = nc.dram_tensor("sin", [S, HD // 2], F16, kind="ExternalInput")
    mask_in = nc.dram_tensor("maskt", [4, P, 512], F16, kind="ExternalInput")
    roff_in = nc.dram_tensor("roff", [1, 2], mybir.dt.uint32, kind="ExternalInput")
    yt_out = nc.dram_tensor("yt_out", [GW, S], F32, kind="ExternalOutput")

    with tile.TileContext(nc) as tc:
        with ExitStack() as ctx:
            const = ctx.enter_context(tc.tile_pool(name="const", bufs=1))
            dram = ctx.enter_context(tc.tile_pool(name="dram", bufs=1, space="DRAM"))

            ident = const.tile([P, P], F16, name="ident")
            make_identity(nc, ident)
            eps_t = const.tile([P, 1], F32, name="eps_t")
            nc.vector.memset(eps_t[:], EPS)
            # k-side: sqrt(sumsq + HD*EPS) = sqrt(HD)*sqrt(mean+EPS), so its
            # reciprocal is SCALE * rstd_k directly.
            eps_hd_t = const.tile([P, 1], F32, name="eps_hd_t")
            nc.vector.memset(eps_hd_t[:], EPS * HD)
            neg1_t = const.tile([P, 1], F32, name="neg1_t")
            nc.vector.memset(neg1_t[:], -1.0)
            ones_f = const.tile([P, P], F32, name="ones_f")
            nc.vector.memset(ones_f[:], 1.0)
            ones2 = const.tile([P, 2], F16, name="ones2")
            nc.scalar.copy(ones2[:], ones_f[:, 0:2])

            # cos/sin/mask tiles declared here; DMAs issued inside phase A
            # after the startup-critical wq/xt pieces
            cos_sb = const.tile([P, CT, HD // 2], F16, name="cos_sb")
            sin_sb = const.tile([P, CT, HD // 2], F16, name="sin_sb")
            mask_sb = const.tile([P, 4, 512], F16, name="mask_sb")

            yt_ics = [dram.tile([GW, 512], F16, name=f"yt_ic{i}") for i in range(ICH)]
            # 4-core groups can't use Shared-output collectives, and Local
            # output hangs at runtime here, so gather across all 8 cores and
            # dynamic-slice the batch's half in phase D.
            ag_ics = [
                dram.tile([NCORES * GW, 512], F16, name=f"ag_ic{i}", addr_space="Shared")
                for i in range(ICH - 1)
            ]
            # last chunk gathers in two head-pair halves so the first half
            # overlaps the attention tail
            ag_last = [
                dram.tile([NCORES * GW // 2, 512], F16, name=f"ag_l{i}", addr_space="Shared")
                for i in range(2)
            ]

            # pools that persist from phase A into B/D
            persist = ctx.enter_context(tc.tile_pool(name="persist", bufs=1))
            wo_sb = persist.tile([P, CT, GW], F16, name="wo_sb")
            qt_sb = persist.tile([P, NHL, S], F16, name="qt_sb")
            kt_sb = persist.tile([P, NHL, S], F16, name="kt_sb")
            v_sb = persist.tile([P, CT, GW], F16, name="v_sb")
            # SCALE * rstd_k per (position-block, head), consumed as the exp scale
            rstdk_sb = persist.tile([P, CT, NHL], F32, name="rstdk_sb")

            # ---------------- Phase A: Q, K, V in one x pass ----------------
            with ExitStack() as pha:
                wpool = pha.enter_context(tc.tile_pool(name="wpool", bufs=1))
                xt_pool = pha.enter_context(tc.tile_pool(name="xt_pool", bufs=4))
                proj_ps = pha.enter_context(tc.tile_pool(name="proj_ps", bufs=2, space="PSUM"))
                tp_ps = pha.enter_context(tc.tile_pool(name="tp_ps", bufs=2, space="PSUM"))
                rope = pha.enter_context(tc.tile_pool(name="rope", bufs=2))
                stat = pha.enter_context(tc.tile_pool(name="stat", bufs=2))

                wq_sb = wpool.tile([P, CT, GW], F16, name="wq_sb")
                wk_sb = wpool.tile([P, CT, GW], F16, name="wk_sb")
                wv_sb = wpool.tile([P, CT, GW], F16, name="wv_sb")

                def load_w(dst, src, splits=((0, 8), (8, 16))):
                    # split loads: finer deps + DMA engine parallelism
                    for c0, c1 in splits:
                        nc.sync.dma_start(
                            out=dst[:, c0:c1, :],
                            in_=src.ap()[c0 * P:c1 * P, :]
                                .rearrange("(a p) f -> p a f", p=P),
                        )

                FIRST = ((0, 1), (1, 4), (4, 10), (10, 16))
                QUARTS = ((0, 4), (4, 8), (8, 12), (12, 16))

                def load_xt(xt_ch, ica, splits):
                    for c0, c1 in splits:
                        nc.sync.dma_start(
                            out=xt_ch[:, c0:c1, :],
                            in_=xt_in.ap()[c0 * P:c1 * P, ica * 256:(ica + 1) * 256]
                                .rearrange("(a p) f -> p a f", p=P),
                        )

                # startup-critical interleave: first wq piece, first xt piece,
                # first cos/sin rows, then the rest
                nc.sync.dma_start(
                    out=wq_sb[:, 0:1, :],
                    in_=wq_in.ap()[0:P, :].rearrange("(a p) f -> p a f", p=P))
                xt_chs = []
                xt0 = xt_pool.tile([P, CT, 256], F16, name="xt_ch0", tag="xt")
                xt_chs.append(xt0)
                nc.sync.dma_start(
                    out=xt0[:, 0:1, :],
                    in_=xt_in.ap()[0:P, 0:256].rearrange("(a p) f -> p a f", p=P))
                nc.sync.dma_start(
                    out=cos_sb[:, 0:2, :],
                    in_=cos_in.ap()[0:2 * P, :].rearrange("(a p) f -> p a f", p=P))
                nc.sync.dma_start(
                    out=sin_sb[:, 0:2, :],
                    in_=sin_in.ap()[0:2 * P, :].rearrange("(a p) f -> p a f", p=P))
                load_w(wq_sb, wq_in, ((1, 4), (4, 10), (10, 16)))
                load_xt(xt0, 0, ((1, 4), (4, 10), (10, 16)))
                nc.sync.dma_start(
                    out=cos_sb[:, 2:CT, :],
                    in_=cos_in.ap()[2 * P:CT * P, :].rearrange("(a p) f -> p a f", p=P))
                nc.sync.dma_start(
                    out=sin_sb[:, 2:CT, :],
                    in_=sin_in.ap()[2 * P:CT * P, :].rearrange("(a p) f -> p a f", p=P))

                for ica in range(8):
                    if ica > 0:
                        xt_ch = xt_pool.tile([P, CT, 256], F16, name=f"xt_ch{ica}", tag="xt")
                        xt_chs.append(xt_ch)
                        load_xt(xt_ch, ica, QUARTS)
                    else:
                        xt_ch = xt0
                    if ica == 0:
                        load_w(wk_sb, wk_in)
                        load_w(wv_sb, wv_in)
                        load_w(wo_sb, wo_in)
                        nc.sync.dma_start(
                            out=mask_sb[:], in_=mask_in.ap().rearrange("t p f -> p t f"))

                    for ib in range(2):
                        ibg = ica * 2 + ib         # global i-block
                        # --- V projection: stays [positions, head-cols] ---
                        vps = proj_ps.tile([P, GW], F32, name=f"vps{ibg}", tag="vps")
                        for ct in range(CT):
                            nc.tensor.matmul(
                                vps[:],
                                xt_ch[:, ct, ib * P:(ib + 1) * P],
                                wv_sb[:, ct, :],
                                start=(ct == 0),
                                stop=(ct == CT - 1),
                            )
                        nc.scalar.copy(v_sb[:, ibg, :], vps[:])

                        # --- Q and K projections ---
                        qps = proj_ps.tile([P, GW], F32, name=f"qps{ibg}", tag="qps")
                        kps = proj_ps.tile([P, GW], F32, name=f"kps{ibg}", tag="kps")
                        for ps, wsb in ((qps, wq_sb), (kps, wk_sb)):
                            for ct in range(CT):
                                nc.tensor.matmul(
                                    ps[:],
                                    xt_ch[:, ct, ib * P:(ib + 1) * P],
                                    wsb[:, ct, :],
                                    start=(ct == 0),
                                    stop=(ct == CT - 1),
                                )
                        # qk: [P, 2, GW] = q then k, processed jointly
                        qk = rope.tile([P, 2, GW], F16, name=f"qk{ibg}", tag="qk")
                        nc.scalar.copy(qk[:, 0, :], qps[:])
                        nc.scalar.copy(qk[:, 1, :], kps[:])

                        # rms-norm stats
                        sq = rope.tile([P, 2, GW], F16, name=f"sq{ibg}", tag="sq")
                        rstd = stat.tile([P, 2 * NHL], F32, name=f"rstd{ibg}", tag="rstd")
                        qk8 = qk[:].rearrange("p a (h d) -> p (a h) d", d=HD)
                        sq8 = sq[:].rearrange("p a (h d) -> p (a h) d", d=HD)
                        if USE_TTR:
                            for j in range(2 * NHL):
                                nc.vector.tensor_tensor_reduce(
                                    out=sq8[:, j, :],
                                    in0=qk8[:, j, :],
                                    in1=qk8[:, j, :],
                                    scale=1.0,
                                    scalar=0.0,
                                    op0=mybir.AluOpType.mult,
                                    op1=mybir.AluOpType.add,
                                    accum_out=rstd[:, j:j + 1],
                                )
                        else:
                            nc.vector.tensor_mul(sq[:], qk[:], qk[:])
                            for j in range(2 * NHL):
                                nc.vector.reduce_sum(
                                    rstd[:, j:j + 1],
                                    sq8[:, j, :],
                                    axis=mybir.AxisListType.X,
                                )
                        # q half: sqrt(mean+eps); k half: sqrt(sumsq+HD*eps)
                        nc.scalar.activation(
                            rstd[:, 0:NHL], rstd[:, 0:NHL],
                            mybir.ActivationFunctionType.Sqrt,
                            bias=eps_t[:], scale=1.0 / HD,
                        )
                        nc.scalar.activation(
                            rstd[:, NHL:2 * NHL], rstd[:, NHL:2 * NHL],
                            mybir.ActivationFunctionType.Sqrt,
                            bias=eps_hd_t[:], scale=1.0,
                        )
                        nc.vector.reciprocal(rstd[:], rstd[:])
                        if USE_EXP_SCALE_AP:
                            nc.scalar.copy(rstdk_sb[:, ibg, :], rstd[:, NHL:2 * NHL])

                        # rope on q and k jointly: 8 (head, q/k) rows of [P, HD]
                        qr = rope.tile([P, 2, GW], F16, name=f"qr{ibg}", tag="qr")
                        qr8 = qr[:].rearrange("p a (h d) -> p (a h) d", d=HD)
                        tmp = rope.tile([P, 2 * NHL, HD // 2], F16, name=f"tmp{ibg}", tag="tmp")
                        cosB = cos_sb[:, ibg:ibg + 1, :].broadcast_to((P, 2 * NHL, HD // 2))
                        sinB = sin_sb[:, ibg:ibg + 1, :].broadcast_to((P, 2 * NHL, HD // 2))
                        h1 = qk8[:, :, 0:HD // 2]
                        h2 = qk8[:, :, HD // 2:HD]
                        # r1 = q1*cos + q2*sin ; r2 = q2*cos - q1*sin
                        nc.vector.tensor_mul(qr8[:, :, 0:HD // 2], h1, cosB)
                        nc.vector.tensor_mul(tmp[:], h2, sinB)
                        nc.vector.tensor_add(qr8[:, :, 0:HD // 2], qr8[:, :, 0:HD // 2], tmp[:])
                        nc.vector.tensor_mul(qr8[:, :, HD // 2:HD], h2, cosB)
                        nc.vector.tensor_mul(tmp[:], h1, sinB)
                        nc.vector.tensor_sub(
                            qr8[:, :, HD // 2:HD], qr8[:, :, HD // 2:HD], tmp[:]
                        )
                        # rstd_q applied to q half; k's either folds into the
                        # exp scale or is applied here (k-half rstd already
                        # includes the 1/sqrt(HD) factor)
                        nmul = NHL if USE_EXP_SCALE_AP else 2 * NHL
                        for j in range(nmul):
                            nc.vector.tensor_scalar_mul(
                                qr8[:, j, :],
                                qr8[:, j, :],
                                rstd[:, j:j + 1],
                            )
                        # transpose per (q/k, head) into SBUF-resident qt/kt
                        for a, dst in ((0, qt_sb), (1, kt_sb)):
                            for h in range(NHL):
                                tp = tp_ps.tile([P, P], F16, name=f"tp{ibg}_{a}_{h}", tag="tp")
                                nc.tensor.transpose(tp[:], qr[:, a, h * HD:(h + 1) * HD], ident[:])
                                nc.scalar.copy(dst[:, h, ibg * P:(ibg + 1) * P], tp[:])

            # ------- Phase B: attention (i-chunk outer) + interleaved o_proj -------
            with ExitStack() as phb:
                et_pool = phb.enter_context(tc.tile_pool(name="et_pool", bufs=6))
                s_ps = phb.enter_context(tc.tile_pool(name="s_ps", bufs=3, space="PSUM"))
                acc_ps = phb.enter_context(tc.tile_pool(name="acc_ps", bufs=2, space="PSUM"))
                den_psp = phb.enter_context(tc.tile_pool(name="den_psp", bufs=2, space="PSUM"))
                bsmall = phb.enter_context(tc.tile_pool(name="bsmall", bufs=2))
                ag_pool = phb.enter_context(tc.tile_pool(name="ag_pool", bufs=2))
                d_ps = phb.enter_context(tc.tile_pool(name="d_ps", bufs=1, space="PSUM"))
                ev2 = phb.enter_context(tc.tile_pool(name="ev2", bufs=3))
                roffp = phb.enter_context(tc.tile_pool(name="roffp", bufs=1))

                roff_sb = roffp.tile([1, 2], mybir.dt.uint32, name="roff_sb")
                nc.sync.dma_start(out=roff_sb[:], in_=roff_in[:, :])
                roff_reg = nc.alloc_registers()
                roff_reg2 = nc.alloc_registers()
                nc.regs_load([roff_reg, roff_reg2], roff_sb[0:1, 0:2])
                rv = nc.snap(roff_reg, donate=True)
                rv2 = nc.snap(roff_reg2, donate=True)

                def emit_score(ic, h, jb):
                    """score matmul + exp for one j-block.  On diagonal blocks
                    (key block t), columns < t*128 are fully masked: et there is
                    memset to 0 (off the critical chain), score/exp run only on
                    live columns, and the 1/0 mask multiplies just the 128-col
                    boundary block."""
                    t = jb - 4 * ic
                    c0 = max(t, 0) * P
                    sp = s_ps.tile([P, 512], F32, name=f"s{h}_{ic}_{jb}", tag="s")
                    et = et_pool.tile([P, 512], F16, name=f"et{h}_{ic}_{jb}", tag="et")
                    if c0 > 0:
                        nc.vector.memset(et[:, 0:c0], 0.0)
                    nc.tensor.matmul(
                        sp[:, c0:512],
                        kt_sb[:, h, jb * P:(jb + 1) * P],
                        qt_sb[:, h, ic * 512 + c0:(ic + 1) * 512],
                        start=True, stop=True,
                    )
                    nc.scalar.activation(
                        et[:, c0:512], sp[:, c0:512],
                        mybir.ActivationFunctionType.Exp,
                        bias=neg1_t[:],
                        scale=rstdk_sb[:, jb, h:h + 1] if USE_EXP_SCALE_AP else 1.0,
                    )
                    if t >= 0:
                        # boundary 128-col block is triangular
                        nc.vector.tensor_mul(
                            et[:, c0:c0 + P], et[:, c0:c0 + P],
                            mask_sb[:, t, c0:c0 + P],
                        )
                    return et

                def emit_oproj(icc):
                    """o_proj for one gathered 512-position chunk."""
                    ag_ch = ag_pool.tile([P, CT, 512], F16, name=f"ag{icc}", tag="ag")
                    if icc < ICH - 1:
                        # 4 quarter-loads: spread across DMA engines, finer deps
                        for quart in range(4):
                            c0, c1 = quart * (CT // 4), (quart + 1) * (CT // 4)
                            nc.sync.dma_start(
                                out=ag_ch[:, c0:c1, :],
                                in_=ag_ics[icc][bass.ds(rv, D), :]
                                    .rearrange("(t p) f -> p t f", p=P)[:, c0:c1, :],
                            )
                        mts = list(range(CT))
                    else:
                        # last chunk arrives as two head-pair half-gathers;
                        # order the contraction so the first half's work can
                        # start before the second half lands
                        ag4 = ag_ch[:].rearrange("p (g e) f -> p g e f", e=2)
                        for half in range(2):
                            nc.sync.dma_start(
                                out=ag4[:, :, half, :],
                                in_=ag_last[half][bass.ds(rv2, D // 2), :]
                                    .rearrange("(g p) f -> p g f", p=P),
                            )
                        # ag_ch[:, t] now holds ct = g*4 + half*2 + (t%2)... see
                        # below: position t=(g,e) maps to head hl = 2*half+e
                        mts = None
                    for oc in range(4):
                        y_ps = d_ps.tile([P, 512], F32, name=f"yp{icc}_{oc}", tag="yp")
                        if mts is not None:
                            order = mts
                            src = lambda mt: wo_sb[:, mt, oc * P:(oc + 1) * P]
                        else:
                            # half-gather `half` lands in slots g*4 + 2e + half
                            # and holds head hl = 2*half+e (ct = g*4+hl); run
                            # half 0's slots first
                            order = [(half, g, e) for half in range(2)
                                     for g in range(4) for e in range(2)]
                            src = None
                        n = len(order)
                        for k, item in enumerate(order):
                            if mts is not None:
                                mt = item
                                wo_ap = wo_sb[:, mt, oc * P:(oc + 1) * P]
                                ag_ap = ag_ch[:, mt, :]
                            else:
                                half, g, e = item
                                ct = g * 4 + 2 * half + e
                                wo_ap = wo_sb[:, ct, oc * P:(oc + 1) * P]
                                slot = g * 4 + 2 * e + half
                                ag_ap = ag_ch[:, slot, :]
                            nc.tensor.matmul(
                                y_ps[:], wo_ap, ag_ap,
                                start=(k == 0), stop=(k == n - 1),
                            )
                        y_sb = ev2.tile([P, 512], F32, name=f"ysb{icc}_{oc}", tag="ysb")
                        nc.scalar.copy(y_sb[:], y_ps[:])
                        nc.scalar.dma_start(
                            out=yt_out[oc * P:(oc + 1) * P, icc * 512:(icc + 1) * 512],
                            in_=y_sb[:],
                        )

                for ic in range(ICH):
                    njb = 4 * ic + 4
                    for h in range(NHL):
                        yt_ps = acc_ps.tile([P, 512], F32, name=f"yt{h}_{ic}", tag="yt")
                        den_ps = den_psp.tile([2, 512], F32, name=f"den{h}_{ic}", tag="den")
                        # software-pipelined: scores run 2 j-blocks ahead so the
                        # PE never head-of-line blocks on exp(jb)
                        ets = {0: emit_score(ic, h, 0)}
                        if njb > 1:
                            ets[1] = emit_score(ic, h, 1)
                        for jb in range(njb):
                            if jb + 2 < njb:
                                ets[jb + 2] = emit_score(ic, h, jb + 2)
                            et = ets.pop(jb)
                            t = jb - 4 * ic
                            # middle diagonal blocks only touch live columns;
                            # first/last must span the full accumulation region
                            c0 = 0 if (t < 1 or jb == njb - 1) else t * P
                            nc.tensor.matmul(
                                yt_ps[:, c0:512],
                                v_sb[:, jb, h * HD:(h + 1) * HD],
                                et[:, c0:512],
                                start=(jb == 0), stop=(jb == njb - 1),
                            )
                            nc.tensor.matmul(
                                den_ps[:, c0:512],
                                ones2[:],
                                et[:, c0:512],
                                start=(jb == 0), stop=(jb == njb - 1),
                            )
                        rden = bsmall.tile([1, 512], F32, name=f"rd{h}_{ic}", tag="rden")
                        nc.vector.reciprocal_approx_fast(rden[:], den_ps[0:1, :])
                        # broadcast 1/den across partitions on the idle gpsimd
                        # engine (no PE matmul, no PSUM bank)
                        bc_sb = bsmall.tile([P, 512], F32, name=f"bcs{h}_{ic}", tag="bcs")
                        nc.gpsimd.partition_broadcast(bc_sb[:], rden[:], channels=P)
                        yt_sb = bsmall.tile([P, 512], F16, name=f"yts{h}_{ic}", tag="yts")
                        nc.vector.tensor_mul(yt_sb[:], yt_ps[:], bc_sb[:])
                        nc.gpsimd.dma_start(
                            out=yt_ics[ic][h * P:(h + 1) * P, :],
                            in_=yt_sb[:],
                        )
                        if ic == ICH - 1 and h % 2 == 1:
                            # last chunk: gather each head pair as soon as it
                            # is done, overlapping the attention tail
                            nc.gpsimd.collective_compute(
                                "AllGather",
                                mybir.AluOpType.bypass,
                                replica_groups=[list(range(NCORES))],
                                ins=[yt_ics[ic][(h - 1) * P:(h + 1) * P, :].opt()],
                                outs=[ag_last[h // 2][:].opt()],
                            )
                    # per-chunk AllGather; fires as soon as chunk ic is written
                    if ic < ICH - 1:
                        nc.gpsimd.collective_compute(
                            "AllGather",
                            mybir.AluOpType.bypass,
                            replica_groups=[list(range(NCORES))],
                            ins=[yt_ics[ic][:].opt()],
                            outs=[ag_ics[ic][:].opt()],
                        )
                    # o_proj for chunk ic-2: its AllGather completed during the
                    # previous attention chunk, so no PE head-of-line stall
                    if ic >= 2:
                        with tc.tile_wait_until(0.26 + 0.03 * (ic - 2)):
                            emit_oproj(ic - 2)
                with tc.tile_wait_until(0.32):
                    emit_oproj(ICH - 2)
                with tc.tile_wait_until(0.36):
                    emit_oproj(ICH - 1)

    nc.compile()
    _program_cache["nc"] = nc
    return nc


def _rope_tables():
    inv_freq = 1.0 / (10000.0 ** (np.arange(0, HD, 2, dtype=np.float32) / HD))
    pos = np.arange(S, dtype=np.float32)
    freqs = np.outer(pos, inv_freq).astype(np.float32)
    return np.cos(freqs).astype(np.float16), np.sin(freqs).astype(np.float16)


def _mask_tiles():
    m = np.zeros((4, P, 512), dtype=np.float16)
    jj = np.arange(P)[:, None]
    ii = np.arange(512)[None, :]
    for t in range(4):
        m[t] = np.where(t * P + jj > ii, 0.0, 1.0)
    return m


def make_in_maps(x, Wq, Wk, Wv, Wo):
    x = np.asarray(x, dtype=np.float32)
    cos, sin = _rope_tables()
    maskt = _mask_tiles()
    wqT = np.ascontiguousarray(np.asarray(Wq, dtype=np.float32).T.astype(np.float16))
    wkT = np.ascontiguousarray(np.asarray(Wk, dtype=np.float32).T.astype(np.float16))
    wvT = np.ascontiguousarray(np.asarray(Wv, dtype=np.float32).T.astype(np.float16))
    woT = np.ascontiguousarray(np.asarray(Wo, dtype=np.float32).T.astype(np.float16))
    xts = [np.ascontiguousarray(x[b].T.astype(np.float16)) for b in range(2)]
    in_maps = []
    for c in range(NCORES):
        b, g = c // 4, c % 4
        sl = slice(g * GW, (g + 1) * GW)
        in_maps.append({
            "roff": np.array([[b * D, b * (D // 2)]], dtype=np.uint32),
            "xt": xts[b],
            "wq": np.ascontiguousarray(wqT[:, sl]),
            "wk": np.ascontiguousarray(wkT[:, sl]),
            "wv": np.ascontiguousarray(wvT[:, sl]),
            "wo": np.ascontiguousarray(woT[:, sl]),
            "cos": cos,
            "sin": sin,
            "maskt": maskt,
        })
    return in_maps


def assemble_output(results):
    y = np.empty((2, S, D), dtype=np.float32)
    for c in range(NCORES):
        b, g = c // 4, c % 4
        y[b][:, g * GW:(g + 1) * GW] = results[c]["yt_out"].T
    return y


def kernel(x, Wq, Wk, Wv, Wo):
    nc = build_program()
    in_maps = make_in_maps(x, Wq, Wk, Wv, Wo)
    res = run_bass_kernel_spmd(nc, in_maps, core_ids=list(range(NCORES)))
    return assemble_output(res.results)
